# revision 7
# baseline (speedup 1.0000x reference)
"""Trainium2 Bass kernel for Chn8ActGrp3WgtQuantizedLinear — fp16 XBAR version.

Computes: out = fake_quant8_per_row(x) @ dequant(weight_qvals, weight_scales).T

  x:             (1024, 4096)  f32
  weight_qvals:  (11008, 4096) int32, 3-bit values in [-4, 3]
  weight_scales: (11008, 32)   f32
  out:           (1024, 11008) f32

Strategy (tensor parallel over 8 NeuronCores, shard N -> 1376/core):
  Host: W = dequant(qvals, scales) -> fp16, K-major per core; x -> fp16.
  Device, per m-tile (128 rows):
    stats (min/max, fp16 2x DVE) -> scale sc, inv, zr = MAGIC + zero
    u = x*inv + zr           (one DVE ts; the add rounds RNE to MAGIC + qx)
    a = fp16(u - zr)         (ACT; = qx - zero, integer in [-255,255], exact)
    aT via batched DMA-XBAR transpose (16x128 tiles, no PE involvement)
    psum[m,n] += aT_g.T @ W_g  over 32 k-groups (fp16 matmuls, hidden ldw)
    evict: out = psum * sc  (ACT, fp16 out)
  PE queue is pure matmuls: quant work lives on DVE/ACT/DMA rings.
  Rings: sync = x loads + XBAR transposes; scalar = weights + out stores.
"""

import sys
import types

import numpy as np

M, K, N, GS = 1024, 4096, 11008, 128
NCORES = 8
NC = N // NCORES  # 1376
NGRP = K // GS  # 32
MTILES = M // 128  # 8
XCH = 1024
NXC = K // XCH
MAGIC = 12582912.0  # 1.5 * 2**23

_CACHE = {}
LAST_RESULTS = None


def _install_axon_ntff_hook():
    try:
        if "antenv.axon_hooks" in sys.modules:
            return
        import antenv

        mod = types.ModuleType("antenv.axon_hooks")
        _state = {"hook": None}
        mod.set_axon_ntff_profile_hook = lambda h: _state.__setitem__("hook", h)
        mod.get_axon_ntff_profile_hook = lambda: _state["hook"]
        sys.modules["antenv.axon_hooks"] = mod
        antenv.axon_hooks = mod

        from trn_agent_boot.trn_boot import _ntff_profile_via_ctypes

        mod.set_axon_ntff_profile_hook(
            _ntff_profile_via_ctypes("/opt/axon/libaxon_pjrt.so")
        )
    except Exception:
        pass


def _build():
    if "nc" in _CACHE:
        return _CACHE["nc"]

    import concourse.bass as bass
    import concourse.tile as tile
    from concourse import bacc, mybir

    dt = mybir.dt
    F32, F16 = dt.float32, dt.float16
    ALU = mybir.AluOpType
    ACTF = mybir.ActivationFunctionType
    AX = mybir.AxisListType

    nc = bacc.Bacc("TRN2", target_bir_lowering=False, debug=False,
                   num_devices=NCORES)

    x_d = nc.dram_tensor("x", [M, K], F16, kind="ExternalInput").ap()
    w_d = nc.dram_tensor("w16", [K, NC], F16, kind="ExternalInput").ap()
    out_d = nc.dram_tensor("out", [M, NC], F16, kind="ExternalOutput").ap()

    CHUNKS = [(c, min(512, NC - c)) for c in range(0, NC, 512)]

    with tile.TileContext(nc) as tc:
        import contextlib

        ctx = contextlib.ExitStack()
        with ctx:
            whpool = ctx.enter_context(tc.tile_pool(name="wh", bufs=1))
            xp = ctx.enter_context(tc.tile_pool(name="x", bufs=3))
            up = ctx.enter_context(tc.tile_pool(name="u", bufs=2))
            ap_ = ctx.enter_context(tc.tile_pool(name="a", bufs=2))
            atp = ctx.enter_context(tc.tile_pool(name="at", bufs=2))
            outp = ctx.enter_context(tc.tile_pool(name="o", bufs=2))
            vecs = ctx.enter_context(tc.tile_pool(name="v", bufs=2))
            ps_out = ctx.enter_context(
                tc.tile_pool(name="pso", bufs=2, space="PSUM"))

            # weights [128, 32(g), NC] fp16, streamed group by group
            w = whpool.tile([128, NGRP, NC], F16)
            for g in range(NGRP):
                nc.gpsimd.dma_start(w[:, g, :], w_d[g * 128:(g + 1) * 128, :])

            scp_of = {}
            at_of = {}

            def quant_phase(m):
                x_t = xp.tile([128, K], F16, tag="xt")
                mxp = vecs.tile([128, NXC], F16, tag="mxp")
                mnp = vecs.tile([128, NXC], F16, tag="mnp")
                for j in range(NXC):
                    sl = slice(j * XCH, (j + 1) * XCH)
                    nc.sync.dma_start(x_t[:, sl], x_d[m * 128:(m + 1) * 128, sl])
                    nc.vector.tensor_reduce(mxp[:, j:j + 1], x_t[:, sl],
                                            axis=AX.X, op=ALU.max)
                    nc.vector.tensor_reduce(mnp[:, j:j + 1], x_t[:, sl],
                                            axis=AX.X, op=ALU.min)
                mx = vecs.tile([128, 1], F32, tag="mx")
                nc.vector.tensor_reduce(mx[:], mxp[:], axis=AX.X, op=ALU.max)
                mn = vecs.tile([128, 1], F32, tag="mn")
                nc.vector.tensor_reduce(mn[:], mnp[:], axis=AX.X, op=ALU.min)
                xc = vecs.tile([128, 1], F32, tag="xc")
                nc.vector.tensor_scalar(xc[:], mx[:], 0.0, None, ALU.max)
                nn_ = vecs.tile([128, 1], F32, tag="nn")
                nc.vector.tensor_scalar(nn_[:], mn[:], 0.0, None, ALU.min)
                df = vecs.tile([128, 1], F32, tag="df")
                nc.vector.tensor_tensor(df[:], xc[:], nn_[:], ALU.subtract)
                sc = vecs.tile([128, 1], F32, tag="sc")
                nc.vector.tensor_scalar(sc[:], df[:], 1.0 / 255.0, 1e-9,
                                        ALU.mult, ALU.max)
                inv = vecs.tile([128, 1], F32, tag="inv")
                nc.vector.reciprocal(inv[:], sc[:])
                z0 = vecs.tile([128, 1], F32, tag="z0")
                nc.vector.tensor_tensor(z0[:], nn_[:], inv[:], ALU.mult)
                z1 = vecs.tile([128, 1], F32, tag="z1")
                nc.vector.tensor_scalar(z1[:], z0[:], -1.0, -128.0,
                                        ALU.mult, ALU.add)
                zr = vecs.tile([128, 1], F32, tag="zr")
                nc.vector.tensor_scalar(zr[:], z1[:], MAGIC, None, ALU.add)
                nzr = vecs.tile([128, 1], F32, tag="nzr")
                nc.vector.tensor_scalar(nzr[:], zr[:], -1.0, None, ALU.mult)

                aT = atp.tile([128, NGRP, 128], F16, tag="aT")
                gj = XCH // 128
                for j in range(NXC):
                    sl = slice(j * XCH, (j + 1) * XCH)
                    u = up.tile([128, XCH], F32, tag="u")
                    nc.vector.tensor_scalar(u[:], x_t[:, sl], inv[:], zr[:],
                                            ALU.mult, ALU.add)
                    a_t = ap_.tile([128, XCH], F16, tag="a")
                    nc.scalar.activation(a_t[:], u[:], ACTF.Identity,
                                         bias=nzr[:], scale=1.0)
                    nc.sync.dma_start(aT[:, j * gj:(j + 1) * gj, :], a_t[:],
                                      transpose=True)
                scp_of[m] = sc
                at_of[m] = aT

            def mm_group(psum, aT, g):
                for (c0, cw) in CHUNKS:
                    nc.tensor.matmul(psum[:, c0:c0 + cw],
                                     lhsT=aT[:, g, :],
                                     rhs=w[:, g, c0:c0 + cw],
                                     start=(g == 0), stop=(g == NGRP - 1))

            def mm_phase(m):
                aT = at_of[m]
                psum = ps_out.tile([128, NC], F32, tag="psum")
                for g in range(NGRP):
                    mm_group(psum, aT, g)
                return psum

            def evict_phase(m, psum):
                o_t = outp.tile([128, NC], F16, tag="o")
                nc.scalar.activation(o_t[:], psum[:], ACTF.Identity,
                                     bias=0.0, scale=scp_of[m][:])
                nc.scalar.dma_start(out_d[m * 128:(m + 1) * 128, :], o_t[:])

            quant_phase(0)
            quant_phase(1)
            # fused m0+m1: both consume each weight group as it streams in;
            # m0 runs a few groups solo so its mms start first
            ps0 = ps_out.tile([128, NC], F32, tag="psum")
            ps1 = ps_out.tile([128, NC], F32, tag="psum")
            for g in range(4):
                mm_group(ps0, at_of[0], g)
            for g in range(4):
                mm_group(ps1, at_of[1], g)
            for g in range(4, NGRP):
                mm_group(ps0, at_of[0], g)
                mm_group(ps1, at_of[1], g)
            evict_phase(0, ps0)
            evict_phase(1, ps1)
            quant_phase(2)
            quant_phase(3)
            for m in range(2, MTILES):
                psum = mm_phase(m)
                evict_phase(m, psum)
                if m + 2 < MTILES:
                    quant_phase(m + 2)

    nc.compile()
    _CACHE["nc"] = nc
    return nc


def _host_pack(weight_qvals, weight_scales):
    wq = np.asarray(weight_qvals).astype(np.float32)
    ws = np.asarray(weight_scales, dtype=np.float32)
    Wf = (wq.reshape(N, NGRP, GS) * ws[:, :, None]).reshape(N, K)
    w16 = Wf.astype(np.float16)
    del Wf, wq
    shards = []
    for ci in range(NCORES):
        sl = slice(ci * NC, (ci + 1) * NC)
        shards.append({"w16": np.ascontiguousarray(w16[sl].T)})
    return shards


def kernel(x, weight_qvals, weight_scales, group_size):
    global LAST_RESULTS
    _install_axon_ntff_hook()
    from concourse.bass_utils import run_bass_kernel_spmd

    x = np.asarray(x, dtype=np.float32)
    assert int(group_size) == GS
    assert x.shape == (M, K)

    nc = _build()
    shards = _host_pack(weight_qvals, weight_scales)
    x16 = x.astype(np.float16)

    in_maps = []
    for ci in range(NCORES):
        d = {"x": x16}
        d.update(shards[ci])
        in_maps.append(d)

    res = run_bass_kernel_spmd(nc, in_maps, core_ids=list(range(NCORES)))
    LAST_RESULTS = res
    out = np.concatenate(
        [r["out"].astype(np.float32) for r in res.results], axis=1)
    return out


if __name__ == "__main__":
    rng = np.random.default_rng(0)
    xv = rng.standard_normal((M, K)).astype(np.float32)
    wqv = rng.integers(-4, 4, (N, K)).astype(np.int32)
    wsv = (rng.random((N, NGRP)).astype(np.float32) * 0.02 + 1e-4)
    o = kernel(xv, wqv, wsv, GS)
    print("out shape:", o.shape, "finite:", np.isfinite(o).all())


# revision 8
# speedup vs baseline: 1.0614x; 1.0614x over previous
"""Trainium2 Bass kernel for Chn8ActGrp3WgtQuantizedLinear — fp16 XBAR version.

Computes: out = fake_quant8_per_row(x) @ dequant(weight_qvals, weight_scales).T

  x:             (1024, 4096)  f32
  weight_qvals:  (11008, 4096) int32, 3-bit values in [-4, 3]
  weight_scales: (11008, 32)   f32
  out:           (1024, 11008) f32

Strategy (tensor parallel over 8 NeuronCores, shard N -> 1376/core):
  Host: W = dequant(qvals, scales) -> fp16, K-major per core; x -> fp16.
  Device, per m-tile (128 rows):
    stats (min/max, fp16 2x DVE) -> scale sc, inv, zr = MAGIC + zero
    u = x*inv + zr           (one DVE ts; the add rounds RNE to MAGIC + qx)
    a = fp16(u - zr)         (ACT; = qx - zero, integer in [-255,255], exact)
    aT via batched DMA-XBAR transpose (16x128 tiles, no PE involvement)
    psum[m,n] += aT_g.T @ W_g  over 32 k-groups (fp16 matmuls, hidden ldw)
    evict: out = psum * sc  (ACT, fp16 out)
  PE queue is pure matmuls: quant work lives on DVE/ACT/DMA rings.
  Rings: sync = x loads + XBAR transposes; scalar = weights + out stores.
"""

import sys
import types

import numpy as np

M, K, N, GS = 1024, 4096, 11008, 128
NCORES = 8
NC = N // NCORES  # 1376
NGRP = K // GS  # 32
MTILES = M // 128  # 8
XCH = 1024
NXC = K // XCH
MAGIC = 12582912.0  # 1.5 * 2**23

_CACHE = {}
LAST_RESULTS = None


def _install_axon_ntff_hook():
    try:
        if "antenv.axon_hooks" in sys.modules:
            return
        import antenv

        mod = types.ModuleType("antenv.axon_hooks")
        _state = {"hook": None}
        mod.set_axon_ntff_profile_hook = lambda h: _state.__setitem__("hook", h)
        mod.get_axon_ntff_profile_hook = lambda: _state["hook"]
        sys.modules["antenv.axon_hooks"] = mod
        antenv.axon_hooks = mod

        from trn_agent_boot.trn_boot import _ntff_profile_via_ctypes

        mod.set_axon_ntff_profile_hook(
            _ntff_profile_via_ctypes("/opt/axon/libaxon_pjrt.so")
        )
    except Exception:
        pass


def _build():
    if "nc" in _CACHE:
        return _CACHE["nc"]

    import concourse.bass as bass
    import concourse.tile as tile
    from concourse import bacc, mybir

    dt = mybir.dt
    F32, F16 = dt.float32, dt.float16
    ALU = mybir.AluOpType
    ACTF = mybir.ActivationFunctionType
    AX = mybir.AxisListType

    nc = bacc.Bacc("TRN2", target_bir_lowering=False, debug=False,
                   num_devices=NCORES)

    x_d = nc.dram_tensor("x", [M, K], F16, kind="ExternalInput").ap()
    w_d = nc.dram_tensor("w16", [K, NC], F16, kind="ExternalInput").ap()
    out_d = nc.dram_tensor("out", [M, NC], F16, kind="ExternalOutput").ap()

    CHUNKS = [(c, min(512, NC - c)) for c in range(0, NC, 512)]

    with tile.TileContext(nc) as tc:
        import contextlib

        ctx = contextlib.ExitStack()
        with ctx:
            whpool = ctx.enter_context(tc.tile_pool(name="wh", bufs=1))
            xp = ctx.enter_context(tc.tile_pool(name="x", bufs=3))
            up = ctx.enter_context(tc.tile_pool(name="u", bufs=2))
            ap_ = ctx.enter_context(tc.tile_pool(name="a", bufs=2))
            atp = ctx.enter_context(tc.tile_pool(name="at", bufs=2))
            outp = ctx.enter_context(tc.tile_pool(name="o", bufs=2))
            vecs = ctx.enter_context(tc.tile_pool(name="v", bufs=2))
            ps_out = ctx.enter_context(
                tc.tile_pool(name="pso", bufs=2, space="PSUM"))

            # weights [128, 32(g), NC] fp16, streamed group by group
            w = whpool.tile([128, NGRP, NC], F16)

            def load_weights():
                for g in range(NGRP):
                    nc.sync.dma_start(w[:, g, :],
                                      w_d[g * 128:(g + 1) * 128, :])

            scp_of = {}
            at_of = {}
            x_of = {}

            def x_load(m):
                x_t = xp.tile([128, K], F16, tag="xt")
                for j in range(NXC):
                    sl = slice(j * XCH, (j + 1) * XCH)
                    nc.sync.dma_start(x_t[:, sl], x_d[m * 128:(m + 1) * 128, sl])
                x_of[m] = x_t

            def quant_phase(m):
                x_t = x_of[m]
                mxp = vecs.tile([128, NXC], F16, tag="mxp")
                mnp = vecs.tile([128, NXC], F16, tag="mnp")
                for j in range(NXC):
                    sl = slice(j * XCH, (j + 1) * XCH)
                    nc.vector.tensor_reduce(mxp[:, j:j + 1], x_t[:, sl],
                                            axis=AX.X, op=ALU.max)
                    nc.vector.tensor_reduce(mnp[:, j:j + 1], x_t[:, sl],
                                            axis=AX.X, op=ALU.min)
                mx = vecs.tile([128, 1], F32, tag="mx")
                nc.vector.tensor_reduce(mx[:], mxp[:], axis=AX.X, op=ALU.max)
                mn = vecs.tile([128, 1], F32, tag="mn")
                nc.vector.tensor_reduce(mn[:], mnp[:], axis=AX.X, op=ALU.min)
                xc = vecs.tile([128, 1], F32, tag="xc")
                nc.vector.tensor_scalar(xc[:], mx[:], 0.0, None, ALU.max)
                nn_ = vecs.tile([128, 1], F32, tag="nn")
                nc.vector.tensor_scalar(nn_[:], mn[:], 0.0, None, ALU.min)
                df = vecs.tile([128, 1], F32, tag="df")
                nc.vector.tensor_tensor(df[:], xc[:], nn_[:], ALU.subtract)
                sc = vecs.tile([128, 1], F32, tag="sc")
                nc.vector.tensor_scalar(sc[:], df[:], 1.0 / 255.0, 1e-9,
                                        ALU.mult, ALU.max)
                inv = vecs.tile([128, 1], F32, tag="inv")
                nc.vector.reciprocal(inv[:], sc[:])
                z0 = vecs.tile([128, 1], F32, tag="z0")
                nc.vector.tensor_tensor(z0[:], nn_[:], inv[:], ALU.mult)
                z1 = vecs.tile([128, 1], F32, tag="z1")
                nc.vector.tensor_scalar(z1[:], z0[:], -1.0, -128.0,
                                        ALU.mult, ALU.add)
                zr = vecs.tile([128, 1], F32, tag="zr")
                nc.vector.tensor_scalar(zr[:], z1[:], MAGIC, None, ALU.add)
                nzr = vecs.tile([128, 1], F32, tag="nzr")
                nc.vector.tensor_scalar(nzr[:], zr[:], -1.0, None, ALU.mult)

                aT = atp.tile([128, NGRP, 128], F16, tag="aT")
                gj = XCH // 128
                for j in range(NXC):
                    sl = slice(j * XCH, (j + 1) * XCH)
                    u = up.tile([128, XCH], F32, tag="u")
                    nc.vector.tensor_scalar(u[:], x_t[:, sl], inv[:], zr[:],
                                            ALU.mult, ALU.add)
                    a_t = ap_.tile([128, XCH], F16, tag="a")
                    nc.scalar.activation(a_t[:], u[:], ACTF.Identity,
                                         bias=nzr[:], scale=1.0)
                    nc.scalar.dma_start(aT[:, j * gj:(j + 1) * gj, :], a_t[:],
                                        transpose=True)
                scp_of[m] = sc
                at_of[m] = aT

            def mm_group(psum, aT, g):
                for (c0, cw) in CHUNKS:
                    nc.tensor.matmul(psum[:, c0:c0 + cw],
                                     lhsT=aT[:, g, :],
                                     rhs=w[:, g, c0:c0 + cw],
                                     start=(g == 0), stop=(g == NGRP - 1))

            def mm_phase(m):
                aT = at_of[m]
                psum = ps_out.tile([128, NC], F32, tag="psum")
                for g in range(NGRP):
                    mm_group(psum, aT, g)
                return psum

            def evict_phase(m, psum):
                o_t = outp.tile([128, NC], F16, tag="o")
                nc.scalar.activation(o_t[:], psum[:], ACTF.Identity,
                                     bias=0.0, scale=scp_of[m][:])
                nc.scalar.dma_start(out_d[m * 128:(m + 1) * 128, :], o_t[:])

            x_load(0)
            x_load(1)
            load_weights()
            quant_phase(0)
            quant_phase(1)
            # fused m0+m1: both consume each weight group as it streams in;
            # m0 runs a few groups solo so its mms start first
            ps0 = ps_out.tile([128, NC], F32, tag="psum")
            ps1 = ps_out.tile([128, NC], F32, tag="psum")
            for g in range(4):
                mm_group(ps0, at_of[0], g)
            for g in range(4):
                mm_group(ps1, at_of[1], g)
            for g in range(4, NGRP):
                mm_group(ps0, at_of[0], g)
                mm_group(ps1, at_of[1], g)
            evict_phase(0, ps0)
            evict_phase(1, ps1)
            x_load(2)
            quant_phase(2)
            x_load(3)
            quant_phase(3)
            for m in range(2, MTILES):
                psum = mm_phase(m)
                evict_phase(m, psum)
                if m + 2 < MTILES:
                    x_load(m + 2)
                    quant_phase(m + 2)

    nc.compile()
    _CACHE["nc"] = nc
    return nc


def _host_pack(weight_qvals, weight_scales):
    wq = np.asarray(weight_qvals).astype(np.float32)
    ws = np.asarray(weight_scales, dtype=np.float32)
    Wf = (wq.reshape(N, NGRP, GS) * ws[:, :, None]).reshape(N, K)
    w16 = Wf.astype(np.float16)
    del Wf, wq
    shards = []
    for ci in range(NCORES):
        sl = slice(ci * NC, (ci + 1) * NC)
        shards.append({"w16": np.ascontiguousarray(w16[sl].T)})
    return shards


def kernel(x, weight_qvals, weight_scales, group_size):
    global LAST_RESULTS
    _install_axon_ntff_hook()
    from concourse.bass_utils import run_bass_kernel_spmd

    x = np.asarray(x, dtype=np.float32)
    assert int(group_size) == GS
    assert x.shape == (M, K)

    nc = _build()
    shards = _host_pack(weight_qvals, weight_scales)
    x16 = x.astype(np.float16)

    in_maps = []
    for ci in range(NCORES):
        d = {"x": x16}
        d.update(shards[ci])
        in_maps.append(d)

    res = run_bass_kernel_spmd(nc, in_maps, core_ids=list(range(NCORES)))
    LAST_RESULTS = res
    out = np.concatenate(
        [r["out"].astype(np.float32) for r in res.results], axis=1)
    return out


if __name__ == "__main__":
    rng = np.random.default_rng(0)
    xv = rng.standard_normal((M, K)).astype(np.float32)
    wqv = rng.integers(-4, 4, (N, K)).astype(np.int32)
    wsv = (rng.random((N, NGRP)).astype(np.float32) * 0.02 + 1e-4)
    o = kernel(xv, wqv, wsv, GS)
    print("out shape:", o.shape, "finite:", np.isfinite(o).all())
